# revision 34
# baseline (speedup 1.0000x reference)
"""Trainium2 Bass kernel for nn_AttentionResidual (sparse_attention).

Computes, for V:(n=8,b=4,s=2048,d=1024), proj:(12,1024), scale:(1024,), block_idx:
    w       = proj[min(block_idx, 11)]
    rms     = sqrt(mean(V^2, axis=-1) + 1e-5)
    logits  = sum_d (w*scale)[d] * V[...,d] / rms
    weights = softmax(logits, axis=n)
    out     = sum_n weights[n] * V[n]                       # (b,s,d)

Sharding: data-parallel over the 8192 (b,s) positions across 8 NeuronCores
(1024 positions per core). proj/scale fold into one d-vector on the host.

Design (fp16 V in [block, pos, n, d] layout; one 2 MiB DMA per 128-position
block). The kernel is bound by the two free-axis reductions (ws-dot on DVE
scalar_tensor_tensor+accum, sum-of-squares mostly on ACT Square+accum),
which no engine does faster than ~1 elem/cycle/partition (DVE STT has no
2x modes; tensor_scalar CACHE_REDUCE measures 1x on HW despite the cost
model's 4x; GPSIMD can neither reduce along the free axis nor touch PSUM).
Measured balance: 58 ACT / 6 DVE sum-of-squares units + ACT-heavy PSUM
drains lands both engines at ~86% busy.
  - softmax stats on [128,8] tiles: ACT Ln/Exp (one table set with
    Square/Copy), DVE max/sum/recip; sume/rs for block q are interleaved
    behind the first DVE reduce unit of block q+1 so DVE never stalls
    waiting on ACT's Exp.
  - weighted sum on the TensorEngine: all 8 diag(e_n) built by a single
    GPSIMD local_scatter into a [128, 8*128] strip; 2x8 accumulating fp16
    matmuls per block. PSUM is TWO [128,512] tiles (one per bank) so each
    drain half waits only on its own bank's matmuls -- the tile framework
    gates readers on whole-tile writers, not overlapping subtiles.
  - PSUM drain (DMA cannot read PSUM): ACT Copy / DVE tensor_scalar with
    the 1/sum(e) softmax normalization folded into the per-partition
    scale; split ACT-heavy to balance the engines.
  - warmup: wt/didx DMAs ride the GPSIMD queue; block 0's V arrives as 8
    SEPARATE per-n tiles (subtile DMA deps don't exist, separate tiles
    do) with block 1's full-tile trigger interleaved after the 4th, so
    the first reduce unit starts ~1.5us in and block 1 lands before DVE
    finishes block 0.
"""

import numpy as np

N, B, S, D = 8, 4, 2048, 1024
NCORES = 8
BS = B * S            # 8192 flattened (b,s) positions
PER = BS // NCORES    # 1024 positions per core
PB = PER // 128       # 8 position blocks per core
ND = N * D            # 8192 (n,d) elements per position
EPS = 1e-5

# Per-(block, n) engine for the sum-of-squares unit. A=ACT Square+accum,
# V=DVE STT+accum. (G=GPSIMD was tried: codegen rejects TensorScalarPtr
# on Pool -- GPSIMD cannot do free-axis reduces, period.) V units come
# FIRST in a block so the ACT Ln never waits on the tail of the DVE batch.
SOS_ENG = ["VAAAAAAA"] * 6 + ["AAAAAAAA"] * 2  # 58A/6V

_cache = {}


def _build():
    import concourse.tile as tile
    from concourse import bacc, mybir

    OP = mybir.AluOpType
    A = mybir.ActivationFunctionType
    X = mybir.AxisListType.X
    f32 = mybir.dt.float32
    f16 = mybir.dt.float16
    f8 = mybir.dt.float8e4

    from concourse.hw_specs import get_activation_tables

    nc = bacc.Bacc(
        "TRN2",
        target_bir_lowering=False,
        debug=False,
        enable_asserts=False,
        num_devices=NCORES,
        enable_partition_id=False,
    )
    v = nc.dram_tensor("v", [PB, 128, ND], f16, kind="ExternalInput").ap()
    wsb = nc.dram_tensor("wsb", [128, D], f16, kind="ExternalInput").ap()
    didx = nc.dram_tensor("didx", [128, N], mybir.dt.int16, kind="ExternalInput").ap()
    o = nc.dram_tensor("o", [PER, D], f16, kind="ExternalOutput").ap()

    # One ACT table set covers Square/Ln/Exp/Copy; pre-place its load so the
    # bacc pass doesn't ping-pong between smaller sets.
    act_set_id = list(get_activation_tables(nc.m.arch).keys()).index(
        "natural_log_exp_and_others"
    )

    with tile.TileContext(nc) as tc:
        with (
            tc.tile_pool(name="v0p", bufs=8) as v0p,
            tc.tile_pool(name="vp", bufs=3) as vp,
            tc.tile_pool(name="wp", bufs=1) as wp,
            tc.tile_pool(name="scrA", bufs=2) as scrA,
            tc.tile_pool(name="scrV", bufs=2) as scrV,
            tc.tile_pool(name="scrG", bufs=2) as scrG,
            tc.tile_pool(name="st", bufs=8) as st,
            tc.tile_pool(name="dg", bufs=3) as dgp,
            tc.tile_pool(name="ac", bufs=3) as ac,
            tc.tile_pool(name="ps", bufs=3, space="PSUM") as ps,
        ):
            nc.scalar.add_instruction(
                mybir.InstLoadActFuncSet(
                    name=nc.get_next_instruction_name(),
                    ins=[],
                    outs=[],
                    act_func_set_id=act_set_id,
                )
            )
            wt = wp.tile([128, D], f16, tag="w")
            didxt = wp.tile([128, N], mybir.dt.int16, tag="didx")
            epsb = wp.tile([128, 1], f32, tag="eps")
            nc.vector.memset(epsb[:], EPS)

            # Skewed software pipeline, one iteration per 128-position
            # block. In-order engine queues mean a dependency ping-pong
            # (ss -> Ln -> y0 -> lg -> nm -> e -> scatter -> matmul ->
            # drain) stalls every engine if issued densely per block;
            # instead each stage is issued one block behind the stage it
            # depends on, so every queued op's inputs are already complete
            # when reached:
            #   iter pp: ACT[e(pp-1)] DVE[sume,rs(pp-1) after 1st unit]
            #            reductions(pp) ACT[Ln,y0(pp)] DVE[lg,nm(pp)]
            #            Pool[scatter(pp-1)] PE[matmuls(pp-1)]
            #            ACT/DVE[drain(pp-2)] DMA[out(pp-2)]
            blk = {}

            def softmax_epilogue(qb):
                # sume/rs for block qb (DVE smalls feeding the drain scale)
                b = blk[qb]
                sume = st.tile([128, 1], f32, tag="sume", name=f"su_{qb}")
                nc.vector.tensor_reduce(sume[:], b["e"][:], X, OP.add)
                rs = st.tile([128, 1], f32, tag="rs", name=f"rs_{qb}")
                nc.vector.reciprocal(rs[:], sume[:])
                b["rs"] = rs

            for pp in range(PB + 2):
                if pp >= 1 and pp - 1 < PB:
                    b = blk[pp - 1]
                    e = st.tile([128, N], f16, tag="e", name=f"e_{pp - 1}")
                    nc.scalar.activation(
                        e[:], b["lg"][:], A.Exp, bias=b["nm"][:]
                    )
                    b["e"] = e
                    if pp >= PB:
                        # drain iteration: no reduction loop to interleave
                        # behind -- issue the epilogue directly
                        softmax_epilogue(pp - 1)
                if pp < PB:
                    sos_eng = SOS_ENG[pp]
                    if pp == 0:
                        # 8 separate per-n tiles: real DMA-completion
                        # granularity for the first block's reduce units
                        nc.gpsimd.dma_start(wt[:], wsb[:])
                        nc.gpsimd.dma_start(didxt[:], didx[:])
                        tsl = [
                            v0p.tile([128, D], f16, tag=f"v0_{q}",
                                     name=f"v0_{q}")
                            for q in range(8)
                        ]
                        t1 = vp.tile([128, ND], f16, tag="v", name="v_1")
                        for q in range(4):
                            nc.sync.dma_start(
                                tsl[q][:], v[0, :, q * D : (q + 1) * D]
                            )
                        # interleave block 1's (whole-tile) transfer so it
                        # lands before DVE finishes block 0
                        nc.sync.dma_start(t1[:], v[1, :, :])
                        for q in range(4, 8):
                            nc.sync.dma_start(
                                tsl[q][:], v[0, :, q * D : (q + 1) * D]
                            )
                    elif pp == 1:
                        t = t1
                    else:
                        t = vp.tile([128, ND], f16, tag="v", name=f"v_{pp}")
                        nc.sync.dma_start(t[:], v[pp, :, :])
                    ss = st.tile([128, N], f32, tag="ss", name=f"ss_{pp}")
                    dot = st.tile([128, N], f32, tag="dot", name=f"dot_{pp}")
                    for n in range(N):
                        vn = tsl[n][:] if pp == 0 else t[:, n * D : (n + 1) * D]
                        if sos_eng[n] == "A":
                            sq = scrA.tile([128, D], f8, tag="sqA")
                            nc.scalar.activation(
                                sq[:], vn, A.Square,
                                accum_out=ss[:, n : n + 1],
                            )
                        elif sos_eng[n] == "G":
                            sq = scrG.tile([128, D], f8, tag="sqG")
                            nc.gpsimd.scalar_tensor_tensor(
                                out=sq[:], in0=vn, scalar=1.0, in1=vn,
                                op0=OP.mult, op1=OP.mult,
                                accum_out=ss[:, n : n + 1],
                            )
                        else:
                            sq = scrV.tile([128, D], f8, tag="sqV")
                            nc.vector.scalar_tensor_tensor(
                                out=sq[:], in0=vn, scalar=1.0, in1=vn,
                                op0=OP.mult, op1=OP.mult,
                                accum_out=ss[:, n : n + 1],
                            )
                        td = scrV.tile([128, D], f8, tag="tdV")
                        nc.vector.scalar_tensor_tensor(
                            out=td[:], in0=vn, scalar=1.0, in1=wt[:],
                            op0=OP.mult, op1=OP.mult,
                            accum_out=dot[:, n : n + 1],
                        )
                        if n == 0 and pp >= 1:
                            # softmax epilogue of the previous block, issued
                            # behind the first DVE unit of this block so the
                            # ACT Exp above has landed by the time DVE gets
                            # here (no stall on the in-order queue)
                            softmax_epilogue(pp - 1)
                    lnt = st.tile([128, N], f32, tag="lnt", name=f"ln_{pp}")
                    nc.scalar.activation(
                        lnt[:], ss[:], A.Ln, bias=epsb[:], scale=1.0 / D
                    )
                    y0 = st.tile([128, N], f32, tag="y0", name=f"y0_{pp}")
                    nc.scalar.activation(y0[:], lnt[:], A.Exp, scale=-0.5)
                    blk[pp] = {
                        "vsl": (tsl if pp == 0 else None), "t": (None if pp == 0 else t),
                        "dot": dot, "y0": y0,
                        "lg": st.tile([128, N], f32, tag="lg", name=f"lg_{pp}"),
                        "nm": st.tile([128, 1], f32, tag="nm", name=f"nm_{pp}"),
                    }
                if pp >= 1 and pp - 1 < PB:
                    b = blk[pp - 1]
                    dgall = dgp.tile(
                        [128, N * 128], f16, tag="dg", name=f"dg_{pp - 1}"
                    )
                    nc.gpsimd.local_scatter(
                        dgall[:], b["e"][:], didxt[:],
                        channels=128, num_elems=N * 128, num_idxs=N,
                    )
                    # one PSUM tile per bank so each drain half gates only
                    # on its own bank's matmuls
                    psh = [
                        ps.tile([128, 512], f32, tag=f"acc{h}",
                                name=f"ps{h}_{pp - 1}")
                        for h in range(2)
                    ]
                    # bank0 fully first so its drain can start while bank1
                    # still accumulates
                    for h in range(2):
                        for n in range(N):
                            if b["vsl"] is not None:
                                tq = b["vsl"][n][:, h * 512 : (h + 1) * 512]
                            else:
                                tq = b["t"][
                                    :, n * D + h * 512 : n * D + (h + 1) * 512
                                ]
                            nc.tensor.matmul(
                                psh[h][:],
                                dgall[:, n * 128 : (n + 1) * 128],
                                tq,
                                start=(n == 0), stop=(n == N - 1),
                            )
                    b["ps"] = psh
                if pp < PB:
                    b = blk[pp]
                    # tiny [128,8] multiply rides GPSIMD (Pool supports
                    # plain Multiply), freeing DVE for the reduce units
                    nc.gpsimd.tensor_mul(b["lg"][:], b["dot"][:], b["y0"][:])
                    nc.vector.tensor_reduce(
                        b["nm"][:], b["lg"][:], X, OP.max, negate=True
                    )
                if pp >= 2:
                    qp = pp - 2
                    b = blk.pop(qp)
                    acc = ac.tile([128, D], f16, tag="acc_sb")
                    psh = b["ps"]
                    # ACT-heavy drain split (measured balance); each half
                    # DMAs as soon as its drain lands
                    nc.scalar.activation(
                        acc[:, 0:512], psh[0][:], A.Copy, scale=b["rs"][:]
                    )
                    nc.sync.dma_start(
                        o[qp * 128 : (qp + 1) * 128, 0:512], acc[:, 0:512]
                    )
                    if qp % 2 == 1:
                        # odd blocks: second half on DVE (balances the
                        # engines; for the last block it also runs in
                        # parallel with ACT's first half, shortening the
                        # exposed tail)
                        nc.vector.tensor_scalar(
                            acc[:, 512:1024], psh[1][:],
                            b["rs"][:], None, OP.mult,
                        )
                    else:
                        nc.scalar.activation(
                            acc[:, 512:1024], psh[1][:],
                            A.Copy, scale=b["rs"][:],
                        )
                    nc.sync.dma_start(
                        o[qp * 128 : (qp + 1) * 128, 512:1024],
                        acc[:, 512:1024],
                    )

    nc.compile()
    return nc


def get_program():
    if "nc" not in _cache:
        _cache["nc"] = _build()
    return _cache["nc"]


def make_in_maps(V, proj, scale, block_idx):
    V = np.asarray(V, dtype=np.float32)
    proj = np.asarray(proj, dtype=np.float32)
    scale = np.asarray(scale, dtype=np.float32)
    idx = min(int(block_idx), proj.shape[0] - 1)
    ws = (proj[idx] * scale).astype(np.float16)
    wsb = np.ascontiguousarray(np.broadcast_to(ws, (128, D)))
    didx = (
        np.arange(N, dtype=np.int16)[None, :] * 128
        + np.arange(128, dtype=np.int16)[:, None]
    ).astype(np.int16)
    # [N, BS, D] -> [NCORES, PB, 128, N, D] fp16
    Vp = (
        V.reshape(N, NCORES, PB, 128, D)
        .transpose(1, 2, 3, 0, 4)
        .astype(np.float16)
    )
    return [
        {
            "v": np.ascontiguousarray(Vp[k]).reshape(PB, 128, ND),
            "wsb": wsb,
            "didx": didx,
        }
        for k in range(NCORES)
    ]


def kernel(V, proj, scale, block_idx):
    from concourse.bass_utils import run_bass_kernel_spmd

    nc = get_program()
    in_maps = make_in_maps(V, proj, scale, block_idx)
    res = run_bass_kernel_spmd(nc, in_maps, core_ids=list(range(NCORES)))
    _cache["last_exec_time_ns"] = res.exec_time_ns
    _cache["last_results"] = res
    out = np.concatenate(
        [res.results[k]["o"].astype(np.float32) for k in range(NCORES)], axis=0
    )
    return out.reshape(B, S, D)


# revision 36
# speedup vs baseline: 1.3505x; 1.3505x over previous
"""Trainium2 Bass kernel for nn_AttentionResidual (sparse_attention).

Computes, for V:(n=8,b=4,s=2048,d=1024), proj:(12,1024), scale:(1024,), block_idx:
    w       = proj[min(block_idx, 11)]
    rms     = sqrt(mean(V^2, axis=-1) + 1e-5)
    logits  = sum_d (w*scale)[d] * V[...,d] / rms
    weights = softmax(logits, axis=n)
    out     = sum_n weights[n] * V[n]                       # (b,s,d)

Sharding: data-parallel over the 8192 (b,s) positions across 8 NeuronCores
(1024 positions per core). proj/scale fold into one d-vector on the host.

Design (fp16 V in [block, pos, n, d] layout; one 2 MiB DMA per 128-position
block; ~112us measured vs 114.5us prior baseline). The kernel is bound by
the two free-axis reductions (ws-dot on DVE scalar_tensor_tensor+accum,
sum-of-squares mostly on ACT Square+accum), which no engine does faster
than ~1 elem/cycle/partition: DVE STT has no 2x modes; tensor_scalar
CACHE_REDUCE measures 1x on HW despite the cost model's 4x; TTR/bn_stats/
pool are all 1x; GPSIMD can neither reduce along the free axis (codegen
rejects TensorScalarPtr on Pool) nor touch PSUM, and putting even a tiny
[128,8] multiply on its Q7 cores costs +42us of chain latency. Two passes
over V are information-theoretically required (ss and dot are independent
functionals), so the ~86us/engine middle is the floor; measured balance:
58 ACT / 6 DVE sum-of-squares units + ACT-heavy PSUM drains lands ACT and
DVE both at ~86% busy. Fixed framework overheads bound the rest: ~6us
preamble (all-engine barriers + tpb_base loads) and ~6us exit (each
engine zeroes its ~51-semaphore pool one op at a time).
  - softmax stats on [128,8] tiles: ACT Ln/Exp (one table set with
    Square/Copy), DVE max/sum/recip; sume/rs for block q are interleaved
    behind the first DVE reduce unit of block q+1 so DVE never stalls
    waiting on ACT's Exp.
  - weighted sum on the TensorEngine: all 8 diag(e_n) built by a single
    GPSIMD local_scatter into a [128, 8*128] strip; 2x8 accumulating fp16
    matmuls per block. PSUM is TWO [128,512] tiles (one per bank) so each
    drain half waits only on its own bank's matmuls -- the tile framework
    gates readers on whole-tile writers, not overlapping subtiles.
  - PSUM drain (DMA cannot read PSUM): ACT Copy / DVE tensor_scalar with
    the 1/sum(e) softmax normalization folded into the per-partition
    scale; split ACT-heavy to balance the engines.
  - warmup: wt/didx DMAs ride the GPSIMD queue; block 0's V arrives as 8
    SEPARATE per-n tiles (subtile DMA deps don't exist, separate tiles
    do) with block 1's full-tile trigger interleaved after the 4th, so
    the first reduce unit starts ~1.5us in and block 1 lands before DVE
    finishes block 0.
"""

import numpy as np

N, B, S, D = 8, 4, 2048, 1024
NCORES = 8
BS = B * S            # 8192 flattened (b,s) positions
PER = BS // NCORES    # 1024 positions per core
PB = PER // 128       # 8 position blocks per core
ND = N * D            # 8192 (n,d) elements per position
EPS = 1e-5

# Per-(block, n) engine for the sum-of-squares unit. A=ACT Square+accum,
# V=DVE STT+accum. (G=GPSIMD was tried: codegen rejects TensorScalarPtr
# on Pool -- GPSIMD cannot do free-axis reduces, period.) V units come
# FIRST in a block so the ACT Ln never waits on the tail of the DVE batch.
SOS_ENG = ["VAAAAAAA"] * 6 + ["AAAAAAAA"] * 2  # 58A/6V

_cache = {}


def _build():
    import concourse.tile as tile
    from concourse import bacc, mybir

    OP = mybir.AluOpType
    A = mybir.ActivationFunctionType
    X = mybir.AxisListType.X
    f32 = mybir.dt.float32
    f16 = mybir.dt.float16
    f8 = mybir.dt.float8e4

    from concourse.hw_specs import get_activation_tables

    nc = bacc.Bacc(
        "TRN2",
        target_bir_lowering=False,
        debug=False,
        enable_asserts=False,
        num_devices=NCORES,
        enable_partition_id=False,
    )
    v = nc.dram_tensor("v", [PB, 128, ND], f16, kind="ExternalInput").ap()
    wsb = nc.dram_tensor("wsb", [128, D], f16, kind="ExternalInput").ap()
    didx = nc.dram_tensor("didx", [128, N], mybir.dt.int16, kind="ExternalInput").ap()
    o = nc.dram_tensor("o", [PER, D], f16, kind="ExternalOutput").ap()

    # One ACT table set covers Square/Ln/Exp/Copy; pre-place its load so the
    # bacc pass doesn't ping-pong between smaller sets.
    act_set_id = list(get_activation_tables(nc.m.arch).keys()).index(
        "natural_log_exp_and_others"
    )

    with tile.TileContext(nc) as tc:
        with (
            tc.tile_pool(name="v0p", bufs=8) as v0p,
            tc.tile_pool(name="vp", bufs=3) as vp,
            tc.tile_pool(name="wp", bufs=1) as wp,
            tc.tile_pool(name="scrA", bufs=2) as scrA,
            tc.tile_pool(name="scrV", bufs=2) as scrV,
            tc.tile_pool(name="scrG", bufs=2) as scrG,
            tc.tile_pool(name="st", bufs=8) as st,
            tc.tile_pool(name="dg", bufs=3) as dgp,
            tc.tile_pool(name="ac", bufs=3) as ac,
            tc.tile_pool(name="ps", bufs=3, space="PSUM") as ps,
        ):
            nc.scalar.add_instruction(
                mybir.InstLoadActFuncSet(
                    name=nc.get_next_instruction_name(),
                    ins=[],
                    outs=[],
                    act_func_set_id=act_set_id,
                )
            )
            wt = wp.tile([128, D], f16, tag="w")
            didxt = wp.tile([128, N], mybir.dt.int16, tag="didx")
            epsb = wp.tile([128, 1], f32, tag="eps")
            nc.vector.memset(epsb[:], EPS)

            # Skewed software pipeline, one iteration per 128-position
            # block. In-order engine queues mean a dependency ping-pong
            # (ss -> Ln -> y0 -> lg -> nm -> e -> scatter -> matmul ->
            # drain) stalls every engine if issued densely per block;
            # instead each stage is issued one block behind the stage it
            # depends on, so every queued op's inputs are already complete
            # when reached:
            #   iter pp: ACT[e(pp-1)] DVE[sume,rs(pp-1) after 1st unit]
            #            reductions(pp) ACT[Ln,y0(pp)] DVE[lg,nm(pp)]
            #            Pool[scatter(pp-1)] PE[matmuls(pp-1)]
            #            ACT/DVE[drain(pp-2)] DMA[out(pp-2)]
            blk = {}

            def softmax_epilogue(qb):
                # sume/rs for block qb (DVE smalls feeding the drain scale)
                b = blk[qb]
                sume = st.tile([128, 1], f32, tag="sume", name=f"su_{qb}")
                nc.vector.tensor_reduce(sume[:], b["e"][:], X, OP.add)
                rs = st.tile([128, 1], f32, tag="rs", name=f"rs_{qb}")
                nc.vector.reciprocal(rs[:], sume[:])
                b["rs"] = rs

            for pp in range(PB + 2):
                if pp >= 1 and pp - 1 < PB:
                    b = blk[pp - 1]
                    e = st.tile([128, N], f16, tag="e", name=f"e_{pp - 1}")
                    nc.scalar.activation(
                        e[:], b["lg"][:], A.Exp, bias=b["nm"][:]
                    )
                    b["e"] = e
                    if pp >= PB:
                        # drain iteration: no reduction loop to interleave
                        # behind -- issue the epilogue directly
                        softmax_epilogue(pp - 1)
                if pp < PB:
                    sos_eng = SOS_ENG[pp]
                    if pp == 0:
                        # 8 separate per-n tiles: real DMA-completion
                        # granularity for the first block's reduce units
                        nc.gpsimd.dma_start(wt[:], wsb[:])
                        nc.gpsimd.dma_start(didxt[:], didx[:])
                        tsl = [
                            v0p.tile([128, D], f16, tag=f"v0_{q}",
                                     name=f"v0_{q}")
                            for q in range(8)
                        ]
                        t1 = vp.tile([128, ND], f16, tag="v", name="v_1")
                        for q in range(4):
                            nc.sync.dma_start(
                                tsl[q][:], v[0, :, q * D : (q + 1) * D]
                            )
                        # interleave block 1's (whole-tile) transfer so it
                        # lands before DVE finishes block 0
                        nc.sync.dma_start(t1[:], v[1, :, :])
                        for q in range(4, 8):
                            nc.sync.dma_start(
                                tsl[q][:], v[0, :, q * D : (q + 1) * D]
                            )
                    elif pp == 1:
                        t = t1
                    else:
                        t = vp.tile([128, ND], f16, tag="v", name=f"v_{pp}")
                        nc.sync.dma_start(t[:], v[pp, :, :])
                    ss = st.tile([128, N], f32, tag="ss", name=f"ss_{pp}")
                    dot = st.tile([128, N], f32, tag="dot", name=f"dot_{pp}")
                    for n in range(N):
                        vn = tsl[n][:] if pp == 0 else t[:, n * D : (n + 1) * D]
                        if sos_eng[n] == "A":
                            sq = scrA.tile([128, D], f8, tag="sqA")
                            nc.scalar.activation(
                                sq[:], vn, A.Square,
                                accum_out=ss[:, n : n + 1],
                            )
                        elif sos_eng[n] == "G":
                            sq = scrG.tile([128, D], f8, tag="sqG")
                            nc.gpsimd.scalar_tensor_tensor(
                                out=sq[:], in0=vn, scalar=1.0, in1=vn,
                                op0=OP.mult, op1=OP.mult,
                                accum_out=ss[:, n : n + 1],
                            )
                        else:
                            sq = scrV.tile([128, D], f8, tag="sqV")
                            nc.vector.scalar_tensor_tensor(
                                out=sq[:], in0=vn, scalar=1.0, in1=vn,
                                op0=OP.mult, op1=OP.mult,
                                accum_out=ss[:, n : n + 1],
                            )
                        td = scrV.tile([128, D], f8, tag="tdV")
                        nc.vector.scalar_tensor_tensor(
                            out=td[:], in0=vn, scalar=1.0, in1=wt[:],
                            op0=OP.mult, op1=OP.mult,
                            accum_out=dot[:, n : n + 1],
                        )
                        if n == 0 and pp >= 1:
                            # softmax epilogue of the previous block, issued
                            # behind the first DVE unit of this block so the
                            # ACT Exp above has landed by the time DVE gets
                            # here (no stall on the in-order queue)
                            softmax_epilogue(pp - 1)
                    lnt = st.tile([128, N], f32, tag="lnt", name=f"ln_{pp}")
                    nc.scalar.activation(
                        lnt[:], ss[:], A.Ln, bias=epsb[:], scale=1.0 / D
                    )
                    y0 = st.tile([128, N], f32, tag="y0", name=f"y0_{pp}")
                    nc.scalar.activation(y0[:], lnt[:], A.Exp, scale=-0.5)
                    blk[pp] = {
                        "vsl": (tsl if pp == 0 else None), "t": (None if pp == 0 else t),
                        "dot": dot, "y0": y0,
                        "lg": st.tile([128, N], f32, tag="lg", name=f"lg_{pp}"),
                        "nm": st.tile([128, 1], f32, tag="nm", name=f"nm_{pp}"),
                    }
                if pp >= 1 and pp - 1 < PB:
                    b = blk[pp - 1]
                    dgall = dgp.tile(
                        [128, N * 128], f16, tag="dg", name=f"dg_{pp - 1}"
                    )
                    nc.gpsimd.local_scatter(
                        dgall[:], b["e"][:], didxt[:],
                        channels=128, num_elems=N * 128, num_idxs=N,
                    )
                    # one PSUM tile per bank so each drain half gates only
                    # on its own bank's matmuls
                    psh = [
                        ps.tile([128, 512], f32, tag=f"acc{h}",
                                name=f"ps{h}_{pp - 1}")
                        for h in range(2)
                    ]
                    # bank0 fully first so its drain can start while bank1
                    # still accumulates
                    for h in range(2):
                        for n in range(N):
                            if b["vsl"] is not None:
                                tq = b["vsl"][n][:, h * 512 : (h + 1) * 512]
                            else:
                                tq = b["t"][
                                    :, n * D + h * 512 : n * D + (h + 1) * 512
                                ]
                            nc.tensor.matmul(
                                psh[h][:],
                                dgall[:, n * 128 : (n + 1) * 128],
                                tq,
                                start=(n == 0), stop=(n == N - 1),
                            )
                    b["ps"] = psh
                if pp < PB:
                    b = blk[pp]
                    # (GPSIMD tensor_mul here measured +42us: Q7 per-op
                    # latency on the critical softmax chain is brutal --
                    # keep GPSIMD strictly to the off-chain scatter)
                    nc.vector.tensor_mul(b["lg"][:], b["dot"][:], b["y0"][:])
                    nc.vector.tensor_reduce(
                        b["nm"][:], b["lg"][:], X, OP.max, negate=True
                    )
                if pp >= 2:
                    qp = pp - 2
                    b = blk.pop(qp)
                    acc = ac.tile([128, D], f16, tag="acc_sb")
                    psh = b["ps"]
                    # ACT-heavy drain split (measured balance); each half
                    # DMAs as soon as its drain lands
                    nc.scalar.activation(
                        acc[:, 0:512], psh[0][:], A.Copy, scale=b["rs"][:]
                    )
                    nc.sync.dma_start(
                        o[qp * 128 : (qp + 1) * 128, 0:512], acc[:, 0:512]
                    )
                    if qp % 2 == 1:
                        # odd blocks: second half on DVE (balances the
                        # engines; for the last block it also runs in
                        # parallel with ACT's first half, shortening the
                        # exposed tail)
                        nc.vector.tensor_scalar(
                            acc[:, 512:1024], psh[1][:],
                            b["rs"][:], None, OP.mult,
                        )
                    else:
                        nc.scalar.activation(
                            acc[:, 512:1024], psh[1][:],
                            A.Copy, scale=b["rs"][:],
                        )
                    nc.sync.dma_start(
                        o[qp * 128 : (qp + 1) * 128, 512:1024],
                        acc[:, 512:1024],
                    )

    nc.compile()
    return nc


def get_program():
    if "nc" not in _cache:
        _cache["nc"] = _build()
    return _cache["nc"]


def make_in_maps(V, proj, scale, block_idx):
    V = np.asarray(V, dtype=np.float32)
    proj = np.asarray(proj, dtype=np.float32)
    scale = np.asarray(scale, dtype=np.float32)
    idx = min(int(block_idx), proj.shape[0] - 1)
    ws = (proj[idx] * scale).astype(np.float16)
    wsb = np.ascontiguousarray(np.broadcast_to(ws, (128, D)))
    didx = (
        np.arange(N, dtype=np.int16)[None, :] * 128
        + np.arange(128, dtype=np.int16)[:, None]
    ).astype(np.int16)
    # [N, BS, D] -> [NCORES, PB, 128, N, D] fp16
    Vp = (
        V.reshape(N, NCORES, PB, 128, D)
        .transpose(1, 2, 3, 0, 4)
        .astype(np.float16)
    )
    return [
        {
            "v": np.ascontiguousarray(Vp[k]).reshape(PB, 128, ND),
            "wsb": wsb,
            "didx": didx,
        }
        for k in range(NCORES)
    ]


def kernel(V, proj, scale, block_idx):
    from concourse.bass_utils import run_bass_kernel_spmd

    nc = get_program()
    in_maps = make_in_maps(V, proj, scale, block_idx)
    res = run_bass_kernel_spmd(nc, in_maps, core_ids=list(range(NCORES)))
    _cache["last_exec_time_ns"] = res.exec_time_ns
    _cache["last_results"] = res
    out = np.concatenate(
        [res.results[k]["o"].astype(np.float32) for k in range(NCORES)], axis=0
    )
    return out.reshape(B, S, D)


# revision 41
# speedup vs baseline: 1.3661x; 1.0115x over previous
"""Trainium2 Bass kernel for nn_AttentionResidual (sparse_attention).

Computes, for V:(n=8,b=4,s=2048,d=1024), proj:(12,1024), scale:(1024,), block_idx:
    w       = proj[min(block_idx, 11)]
    rms     = sqrt(mean(V^2, axis=-1) + 1e-5)
    logits  = sum_d (w*scale)[d] * V[...,d] / rms
    weights = softmax(logits, axis=n)
    out     = sum_n weights[n] * V[n]                       # (b,s,d)

Sharding: data-parallel over the 8192 (b,s) positions across 8 NeuronCores
(1024 positions per core). proj/scale fold into one d-vector on the host.

Design (fp16 V in [block, pos, n, d] layout; one 2 MiB DMA per 128-position
block; ~112us measured vs 114.5us prior baseline). The kernel is bound by
the two free-axis reductions (ws-dot on DVE scalar_tensor_tensor+accum,
sum-of-squares mostly on ACT Square+accum), which no engine does faster
than ~1 elem/cycle/partition: DVE STT has no 2x modes; tensor_scalar
CACHE_REDUCE measures 1x on HW despite the cost model's 4x; TTR/bn_stats/
pool are all 1x; GPSIMD can neither reduce along the free axis (codegen
rejects TensorScalarPtr on Pool) nor touch PSUM, and putting even a tiny
[128,8] multiply on its Q7 cores costs +42us of chain latency. Two passes
over V are information-theoretically required (ss and dot are independent
functionals), so the ~86us/engine middle is the floor; measured balance:
58 ACT / 6 DVE sum-of-squares units + ACT-heavy PSUM drains lands ACT and
DVE both at ~86% busy. Fixed framework overheads bound the rest: ~6us
preamble (all-engine barriers + tpb_base loads) and ~6us exit (each
engine zeroes its ~51-semaphore pool one op at a time).
  - softmax stats on [128,8] tiles: ACT Ln/Exp (one table set with
    Square/Copy), DVE max/sum/recip; sume/rs for block q are interleaved
    behind the first DVE reduce unit of block q+1 so DVE never stalls
    waiting on ACT's Exp.
  - weighted sum on the TensorEngine: all 8 diag(e_n) built by a single
    GPSIMD local_scatter into a [128, 8*128] strip; 2x8 accumulating fp16
    matmuls per block. PSUM is TWO [128,512] tiles (one per bank) so each
    drain half waits only on its own bank's matmuls -- the tile framework
    gates readers on whole-tile writers, not overlapping subtiles.
  - PSUM drain (DMA cannot read PSUM): ACT Copy / DVE tensor_scalar with
    the 1/sum(e) softmax normalization folded into the per-partition
    scale; split ACT-heavy to balance the engines.
  - warmup: wt/didx DMAs ride the GPSIMD queue; block 0's V arrives as 8
    SEPARATE per-n tiles (subtile DMA deps don't exist, separate tiles
    do) with block 1's full-tile trigger interleaved after the 4th, so
    the first reduce unit starts ~1.5us in and block 1 lands before DVE
    finishes block 0.
"""

import numpy as np

N, B, S, D = 8, 4, 2048, 1024
NCORES = 8
BS = B * S            # 8192 flattened (b,s) positions
PER = BS // NCORES    # 1024 positions per core
PB = PER // 128       # 8 position blocks per core
ND = N * D            # 8192 (n,d) elements per position
EPS = 1e-5

# Per-(block, n) engine for the sum-of-squares unit. A=ACT Square+accum,
# V=DVE STT+accum. (G=GPSIMD was tried: codegen rejects TensorScalarPtr
# on Pool -- GPSIMD cannot do free-axis reduces, period.) V units come
# FIRST in a block so the ACT Ln never waits on the tail of the DVE batch.
SOS_ENG = ["VAAAAAAA"] * 6 + ["AAAAAAAA"] * 2  # 58A/6V

_cache = {}


def _build():
    import concourse.tile as tile
    from concourse import bacc, mybir

    OP = mybir.AluOpType
    A = mybir.ActivationFunctionType
    X = mybir.AxisListType.X
    f32 = mybir.dt.float32
    f16 = mybir.dt.float16
    f8 = mybir.dt.float8e4

    from concourse.hw_specs import get_activation_tables

    nc = bacc.Bacc(
        "TRN2",
        target_bir_lowering=False,
        debug=False,
        enable_asserts=False,
        num_devices=NCORES,
        enable_partition_id=False,
    )
    v = nc.dram_tensor("v", [PB, 128, ND], f16, kind="ExternalInput").ap()
    wsb = nc.dram_tensor("wsb", [128, D], f16, kind="ExternalInput").ap()
    didx = nc.dram_tensor("didx", [128, N], mybir.dt.int16, kind="ExternalInput").ap()
    o = nc.dram_tensor("o", [PER, D], f16, kind="ExternalOutput").ap()

    # One ACT table set covers Square/Ln/Exp/Copy; pre-place its load so the
    # bacc pass doesn't ping-pong between smaller sets.
    act_set_id = list(get_activation_tables(nc.m.arch).keys()).index(
        "natural_log_exp_and_others"
    )

    with tile.TileContext(nc) as tc:
        with (
            tc.tile_pool(name="v0p", bufs=8) as v0p,
            tc.tile_pool(name="vp", bufs=4) as vp,
            tc.tile_pool(name="wp", bufs=1) as wp,
            tc.tile_pool(name="scrA", bufs=2) as scrA,
            tc.tile_pool(name="scrV", bufs=2) as scrV,
            tc.tile_pool(name="scrG", bufs=2) as scrG,
            tc.tile_pool(name="st", bufs=8) as st,
            tc.tile_pool(name="dg", bufs=3) as dgp,
            tc.tile_pool(name="ac", bufs=3) as ac,
            tc.tile_pool(name="ps", bufs=3, space="PSUM") as ps,
        ):
            nc.scalar.add_instruction(
                mybir.InstLoadActFuncSet(
                    name=nc.get_next_instruction_name(),
                    ins=[],
                    outs=[],
                    act_func_set_id=act_set_id,
                )
            )
            wt = wp.tile([128, D], f16, tag="w")
            didxt = wp.tile([128, N], mybir.dt.int16, tag="didx")
            epsb = wp.tile([128, 1], f32, tag="eps")
            nc.vector.memset(epsb[:], EPS)

            # Skewed software pipeline, one iteration per 128-position
            # block. In-order engine queues mean a dependency ping-pong
            # (ss -> Ln -> y0 -> lg -> nm -> e -> scatter -> matmul ->
            # drain) stalls every engine if issued densely per block;
            # instead each stage is issued one block behind the stage it
            # depends on, so every queued op's inputs are already complete
            # when reached:
            #   iter pp: ACT[e(pp-1)] DVE[sume,rs(pp-1) after 1st unit]
            #            reductions(pp) ACT[Ln,y0(pp)] DVE[lg,nm(pp)]
            #            Pool[scatter(pp-1)] PE[matmuls(pp-1)]
            #            ACT/DVE[drain(pp-2)] DMA[out(pp-2)]
            blk = {}

            def softmax_epilogue(qb):
                # sume/rs for block qb (DVE smalls feeding the drain scale)
                b = blk[qb]
                sume = st.tile([128, 1], f32, tag="sume", name=f"su_{qb}")
                nc.vector.tensor_reduce(sume[:], b["e"][:], X, OP.add)
                rs = st.tile([128, 1], f32, tag="rs", name=f"rs_{qb}")
                nc.vector.reciprocal(rs[:], sume[:])
                b["rs"] = rs

            for pp in range(PB + 2):
                if pp >= 1 and pp - 1 < PB:
                    b = blk[pp - 1]
                    e = st.tile([128, N], f16, tag="e", name=f"e_{pp - 1}")
                    nc.scalar.activation(
                        e[:], b["lg"][:], A.Exp, bias=b["nm"][:]
                    )
                    b["e"] = e
                    if pp >= PB:
                        # drain iteration: no reduction loop to interleave
                        # behind -- issue the epilogue directly
                        softmax_epilogue(pp - 1)
                if pp < PB:
                    sos_eng = SOS_ENG[pp]
                    t = vp.tile([128, ND], f16, tag="v", name=f"v_{pp}")
                    if pp == 0:
                        for q in range(4):
                            nc.sync.dma_start(
                                t[:, q * (ND // 4) : (q + 1) * (ND // 4)],
                                v[pp, :, q * (ND // 4) : (q + 1) * (ND // 4)],
                            )
                        nc.sync.dma_start(wt[:], wsb[:])
                        nc.sync.dma_start(didxt[:], didx[:])
                    else:
                        nc.sync.dma_start(t[:], v[pp, :, :])
                    ss = st.tile([128, N], f32, tag="ss", name=f"ss_{pp}")
                    dot = st.tile([128, N], f32, tag="dot", name=f"dot_{pp}")
                    for n in range(N):
                        vn = t[:, n * D : (n + 1) * D]
                        if sos_eng[n] == "A":
                            sq = scrA.tile([128, D], f8, tag="sqA")
                            nc.scalar.activation(
                                sq[:], vn, A.Square,
                                accum_out=ss[:, n : n + 1],
                            )
                        elif sos_eng[n] == "G":
                            sq = scrG.tile([128, D], f8, tag="sqG")
                            nc.gpsimd.scalar_tensor_tensor(
                                out=sq[:], in0=vn, scalar=1.0, in1=vn,
                                op0=OP.mult, op1=OP.mult,
                                accum_out=ss[:, n : n + 1],
                            )
                        else:
                            sq = scrV.tile([128, D], f8, tag="sqV")
                            nc.vector.scalar_tensor_tensor(
                                out=sq[:], in0=vn, scalar=1.0, in1=vn,
                                op0=OP.mult, op1=OP.mult,
                                accum_out=ss[:, n : n + 1],
                            )
                        td = scrV.tile([128, D], f8, tag="tdV")
                        nc.vector.scalar_tensor_tensor(
                            out=td[:], in0=vn, scalar=1.0, in1=wt[:],
                            op0=OP.mult, op1=OP.mult,
                            accum_out=dot[:, n : n + 1],
                        )
                        if n == 0 and pp >= 1:
                            # softmax epilogue of the previous block, issued
                            # behind the first DVE unit of this block so the
                            # ACT Exp above has landed by the time DVE gets
                            # here (no stall on the in-order queue)
                            softmax_epilogue(pp - 1)
                    lnt = st.tile([128, N], f32, tag="lnt", name=f"ln_{pp}")
                    nc.scalar.activation(
                        lnt[:], ss[:], A.Ln, bias=epsb[:], scale=1.0 / D
                    )
                    y0 = st.tile([128, N], f32, tag="y0", name=f"y0_{pp}")
                    nc.scalar.activation(y0[:], lnt[:], A.Exp, scale=-0.5)
                    blk[pp] = {
                        "t": t, "dot": dot, "y0": y0,
                        "lg": st.tile([128, N], f32, tag="lg", name=f"lg_{pp}"),
                        "nm": st.tile([128, 1], f32, tag="nm", name=f"nm_{pp}"),
                    }
                if pp >= 1 and pp - 1 < PB:
                    b = blk[pp - 1]
                    dgall = dgp.tile(
                        [128, N * 128], f16, tag="dg", name=f"dg_{pp - 1}"
                    )
                    nc.gpsimd.local_scatter(
                        dgall[:], b["e"][:], didxt[:],
                        channels=128, num_elems=N * 128, num_idxs=N,
                    )
                    # one PSUM tile per bank so each drain half gates only
                    # on its own bank's matmuls
                    psh = [
                        ps.tile([128, 512], f32, tag=f"acc{h}",
                                name=f"ps{h}_{pp - 1}")
                        for h in range(2)
                    ]
                    # bank0 fully first so its drain can start while bank1
                    # still accumulates
                    tq = b["t"]
                    for h in range(2):
                        for n in range(N):
                            nc.tensor.matmul(
                                psh[h][:],
                                dgall[:, n * 128 : (n + 1) * 128],
                                tq[:, n * D + h * 512 : n * D + (h + 1) * 512],
                                start=(n == 0), stop=(n == N - 1),
                            )
                    b["ps"] = psh
                if pp < PB:
                    b = blk[pp]
                    # (GPSIMD tensor_mul here measured +42us: Q7 per-op
                    # latency on the critical softmax chain is brutal --
                    # keep GPSIMD strictly to the off-chain scatter)
                    nc.vector.tensor_mul(b["lg"][:], b["dot"][:], b["y0"][:])
                    nc.vector.tensor_reduce(
                        b["nm"][:], b["lg"][:], X, OP.max, negate=True
                    )
                if pp >= 2:
                    qp = pp - 2
                    b = blk.pop(qp)
                    acc = ac.tile([128, D], f16, tag="acc_sb")
                    psh = b["ps"]
                    # ACT-heavy drain split (measured balance); each half
                    # DMAs as soon as its drain lands
                    nc.scalar.activation(
                        acc[:, 0:512], psh[0][:], A.Copy, scale=b["rs"][:]
                    )
                    nc.sync.dma_start(
                        o[qp * 128 : (qp + 1) * 128, 0:512], acc[:, 0:512]
                    )
                    if qp % 2 == 1:
                        # odd blocks: second half on DVE (balances the
                        # engines; for the last block it also runs in
                        # parallel with ACT's first half, shortening the
                        # exposed tail)
                        nc.vector.tensor_scalar(
                            acc[:, 512:1024], psh[1][:],
                            b["rs"][:], None, OP.mult,
                        )
                    else:
                        nc.scalar.activation(
                            acc[:, 512:1024], psh[1][:],
                            A.Copy, scale=b["rs"][:],
                        )
                    nc.sync.dma_start(
                        o[qp * 128 : (qp + 1) * 128, 512:1024],
                        acc[:, 512:1024],
                    )

    nc.compile()
    return nc


def get_program():
    if "nc" not in _cache:
        _cache["nc"] = _build()
    return _cache["nc"]


def make_in_maps(V, proj, scale, block_idx):
    V = np.asarray(V, dtype=np.float32)
    proj = np.asarray(proj, dtype=np.float32)
    scale = np.asarray(scale, dtype=np.float32)
    idx = min(int(block_idx), proj.shape[0] - 1)
    ws = (proj[idx] * scale).astype(np.float16)
    wsb = np.ascontiguousarray(np.broadcast_to(ws, (128, D)))
    didx = (
        np.arange(N, dtype=np.int16)[None, :] * 128
        + np.arange(128, dtype=np.int16)[:, None]
    ).astype(np.int16)
    # [N, BS, D] -> [NCORES, PB, 128, N, D] fp16
    Vp = (
        V.reshape(N, NCORES, PB, 128, D)
        .transpose(1, 2, 3, 0, 4)
        .astype(np.float16)
    )
    return [
        {
            "v": np.ascontiguousarray(Vp[k]).reshape(PB, 128, ND),
            "wsb": wsb,
            "didx": didx,
        }
        for k in range(NCORES)
    ]


def kernel(V, proj, scale, block_idx):
    from concourse.bass_utils import run_bass_kernel_spmd

    nc = get_program()
    in_maps = make_in_maps(V, proj, scale, block_idx)
    res = run_bass_kernel_spmd(nc, in_maps, core_ids=list(range(NCORES)))
    _cache["last_exec_time_ns"] = res.exec_time_ns
    _cache["last_results"] = res
    out = np.concatenate(
        [res.results[k]["o"].astype(np.float32) for k in range(NCORES)], axis=0
    )
    return out.reshape(B, S, D)


# revision 42
# speedup vs baseline: 1.3788x; 1.0093x over previous
"""Trainium2 Bass kernel for nn_AttentionResidual (sparse_attention).

Computes, for V:(n=8,b=4,s=2048,d=1024), proj:(12,1024), scale:(1024,), block_idx:
    w       = proj[min(block_idx, 11)]
    rms     = sqrt(mean(V^2, axis=-1) + 1e-5)
    logits  = sum_d (w*scale)[d] * V[...,d] / rms
    weights = softmax(logits, axis=n)
    out     = sum_n weights[n] * V[n]                       # (b,s,d)

Sharding: data-parallel over the 8192 (b,s) positions across 8 NeuronCores
(1024 positions per core). proj/scale fold into one d-vector on the host.

Design (fp16 V in [block, pos, n, d] layout; one 2 MiB DMA per 128-position
block; ~112us measured vs 114.5us prior baseline). The kernel is bound by
the two free-axis reductions (ws-dot on DVE scalar_tensor_tensor+accum,
sum-of-squares mostly on ACT Square+accum), which no engine does faster
than ~1 elem/cycle/partition: DVE STT has no 2x modes; tensor_scalar
CACHE_REDUCE measures 1x on HW despite the cost model's 4x; TTR/bn_stats/
pool are all 1x; GPSIMD can neither reduce along the free axis (codegen
rejects TensorScalarPtr on Pool) nor touch PSUM, and putting even a tiny
[128,8] multiply on its Q7 cores costs +42us of chain latency. Two passes
over V are information-theoretically required (ss and dot are independent
functionals), so the ~86us/engine middle is the floor; measured balance:
58 ACT / 6 DVE sum-of-squares units + ACT-heavy PSUM drains lands ACT and
DVE both at ~86% busy. Fixed framework overheads bound the rest: ~6us
preamble (all-engine barriers + tpb_base loads) and ~6us exit (each
engine zeroes its ~51-semaphore pool one op at a time).
  - softmax stats on [128,8] tiles: ACT Ln/Exp (one table set with
    Square/Copy), DVE max/sum/recip; sume/rs for block q are interleaved
    behind the first DVE reduce unit of block q+1 so DVE never stalls
    waiting on ACT's Exp.
  - weighted sum on the TensorEngine: all 8 diag(e_n) built by a single
    GPSIMD local_scatter into a [128, 8*128] strip; 2x8 accumulating fp16
    matmuls per block. PSUM is TWO [128,512] tiles (one per bank) so each
    drain half waits only on its own bank's matmuls -- the tile framework
    gates readers on whole-tile writers, not overlapping subtiles.
  - PSUM drain (DMA cannot read PSUM): ACT Copy / DVE tensor_scalar with
    the 1/sum(e) softmax normalization folded into the per-partition
    scale; split ACT-heavy to balance the engines.
  - warmup: block 0's V arrives as 4 quarter DMAs. (An 8-separate-tile
    per-n split was tried to start compute earlier -- the ~6us startup
    is actually all-engine-barrier preamble, not DMA wait, and the split
    only added +/-1.5us of schedule variance. This 4-quarter form
    measures 113.3-113.5us with baseline-like ~0.1us repeatability.)
"""

import numpy as np

N, B, S, D = 8, 4, 2048, 1024
NCORES = 8
BS = B * S            # 8192 flattened (b,s) positions
PER = BS // NCORES    # 1024 positions per core
PB = PER // 128       # 8 position blocks per core
ND = N * D            # 8192 (n,d) elements per position
EPS = 1e-5

# Per-(block, n) engine for the sum-of-squares unit. A=ACT Square+accum,
# V=DVE STT+accum. (G=GPSIMD was tried: codegen rejects TensorScalarPtr
# on Pool -- GPSIMD cannot do free-axis reduces, period.) V units come
# FIRST in a block so the ACT Ln never waits on the tail of the DVE batch.
SOS_ENG = ["VAAAAAAA"] * 6 + ["AAAAAAAA"] * 2  # 58A/6V

_cache = {}


def _build():
    import concourse.tile as tile
    from concourse import bacc, mybir

    OP = mybir.AluOpType
    A = mybir.ActivationFunctionType
    X = mybir.AxisListType.X
    f32 = mybir.dt.float32
    f16 = mybir.dt.float16
    f8 = mybir.dt.float8e4

    from concourse.hw_specs import get_activation_tables

    nc = bacc.Bacc(
        "TRN2",
        target_bir_lowering=False,
        debug=False,
        enable_asserts=False,
        num_devices=NCORES,
        enable_partition_id=False,
    )
    v = nc.dram_tensor("v", [PB, 128, ND], f16, kind="ExternalInput").ap()
    wsb = nc.dram_tensor("wsb", [128, D], f16, kind="ExternalInput").ap()
    didx = nc.dram_tensor("didx", [128, N], mybir.dt.int16, kind="ExternalInput").ap()
    o = nc.dram_tensor("o", [PER, D], f16, kind="ExternalOutput").ap()

    # One ACT table set covers Square/Ln/Exp/Copy; pre-place its load so the
    # bacc pass doesn't ping-pong between smaller sets.
    act_set_id = list(get_activation_tables(nc.m.arch).keys()).index(
        "natural_log_exp_and_others"
    )

    with tile.TileContext(nc) as tc:
        with (
            tc.tile_pool(name="v0p", bufs=8) as v0p,
            tc.tile_pool(name="vp", bufs=4) as vp,
            tc.tile_pool(name="wp", bufs=1) as wp,
            tc.tile_pool(name="scrA", bufs=2) as scrA,
            tc.tile_pool(name="scrV", bufs=2) as scrV,
            tc.tile_pool(name="scrG", bufs=2) as scrG,
            tc.tile_pool(name="st", bufs=8) as st,
            tc.tile_pool(name="dg", bufs=3) as dgp,
            tc.tile_pool(name="ac", bufs=3) as ac,
            tc.tile_pool(name="ps", bufs=3, space="PSUM") as ps,
        ):
            nc.scalar.add_instruction(
                mybir.InstLoadActFuncSet(
                    name=nc.get_next_instruction_name(),
                    ins=[],
                    outs=[],
                    act_func_set_id=act_set_id,
                )
            )
            wt = wp.tile([128, D], f16, tag="w")
            didxt = wp.tile([128, N], mybir.dt.int16, tag="didx")
            epsb = wp.tile([128, 1], f32, tag="eps")
            nc.vector.memset(epsb[:], EPS)

            # Skewed software pipeline, one iteration per 128-position
            # block. In-order engine queues mean a dependency ping-pong
            # (ss -> Ln -> y0 -> lg -> nm -> e -> scatter -> matmul ->
            # drain) stalls every engine if issued densely per block;
            # instead each stage is issued one block behind the stage it
            # depends on, so every queued op's inputs are already complete
            # when reached:
            #   iter pp: ACT[e(pp-1)] DVE[sume,rs(pp-1) after 1st unit]
            #            reductions(pp) ACT[Ln,y0(pp)] DVE[lg,nm(pp)]
            #            Pool[scatter(pp-1)] PE[matmuls(pp-1)]
            #            ACT/DVE[drain(pp-2)] DMA[out(pp-2)]
            blk = {}

            def softmax_epilogue(qb):
                # sume/rs for block qb (DVE smalls feeding the drain scale)
                b = blk[qb]
                sume = st.tile([128, 1], f32, tag="sume", name=f"su_{qb}")
                nc.vector.tensor_reduce(sume[:], b["e"][:], X, OP.add)
                rs = st.tile([128, 1], f32, tag="rs", name=f"rs_{qb}")
                nc.vector.reciprocal(rs[:], sume[:])
                b["rs"] = rs

            for pp in range(PB + 2):
                if pp >= 1 and pp - 1 < PB:
                    b = blk[pp - 1]
                    e = st.tile([128, N], f16, tag="e", name=f"e_{pp - 1}")
                    nc.scalar.activation(
                        e[:], b["lg"][:], A.Exp, bias=b["nm"][:]
                    )
                    b["e"] = e
                    if pp >= PB:
                        # drain iteration: no reduction loop to interleave
                        # behind -- issue the epilogue directly
                        softmax_epilogue(pp - 1)
                if pp < PB:
                    sos_eng = SOS_ENG[pp]
                    t = vp.tile([128, ND], f16, tag="v", name=f"v_{pp}")
                    if pp == 0:
                        for q in range(4):
                            nc.sync.dma_start(
                                t[:, q * (ND // 4) : (q + 1) * (ND // 4)],
                                v[pp, :, q * (ND // 4) : (q + 1) * (ND // 4)],
                            )
                        nc.sync.dma_start(wt[:], wsb[:])
                        nc.sync.dma_start(didxt[:], didx[:])
                    else:
                        nc.sync.dma_start(t[:], v[pp, :, :])
                    ss = st.tile([128, N], f32, tag="ss", name=f"ss_{pp}")
                    dot = st.tile([128, N], f32, tag="dot", name=f"dot_{pp}")
                    for n in range(N):
                        vn = t[:, n * D : (n + 1) * D]
                        if sos_eng[n] == "A":
                            sq = scrA.tile([128, D], f8, tag="sqA")
                            nc.scalar.activation(
                                sq[:], vn, A.Square,
                                accum_out=ss[:, n : n + 1],
                            )
                        elif sos_eng[n] == "G":
                            sq = scrG.tile([128, D], f8, tag="sqG")
                            nc.gpsimd.scalar_tensor_tensor(
                                out=sq[:], in0=vn, scalar=1.0, in1=vn,
                                op0=OP.mult, op1=OP.mult,
                                accum_out=ss[:, n : n + 1],
                            )
                        else:
                            sq = scrV.tile([128, D], f8, tag="sqV")
                            nc.vector.scalar_tensor_tensor(
                                out=sq[:], in0=vn, scalar=1.0, in1=vn,
                                op0=OP.mult, op1=OP.mult,
                                accum_out=ss[:, n : n + 1],
                            )
                        td = scrV.tile([128, D], f8, tag="tdV")
                        nc.vector.scalar_tensor_tensor(
                            out=td[:], in0=vn, scalar=1.0, in1=wt[:],
                            op0=OP.mult, op1=OP.mult,
                            accum_out=dot[:, n : n + 1],
                        )
                        if n == 0 and pp >= 1:
                            # softmax epilogue of the previous block, issued
                            # behind the first DVE unit of this block so the
                            # ACT Exp above has landed by the time DVE gets
                            # here (no stall on the in-order queue)
                            softmax_epilogue(pp - 1)
                    lnt = st.tile([128, N], f32, tag="lnt", name=f"ln_{pp}")
                    nc.scalar.activation(
                        lnt[:], ss[:], A.Ln, bias=epsb[:], scale=1.0 / D
                    )
                    y0 = st.tile([128, N], f32, tag="y0", name=f"y0_{pp}")
                    nc.scalar.activation(y0[:], lnt[:], A.Exp, scale=-0.5)
                    blk[pp] = {
                        "t": t, "dot": dot, "y0": y0,
                        "lg": st.tile([128, N], f32, tag="lg", name=f"lg_{pp}"),
                        "nm": st.tile([128, 1], f32, tag="nm", name=f"nm_{pp}"),
                    }
                if pp >= 1 and pp - 1 < PB:
                    b = blk[pp - 1]
                    dgall = dgp.tile(
                        [128, N * 128], f16, tag="dg", name=f"dg_{pp - 1}"
                    )
                    nc.gpsimd.local_scatter(
                        dgall[:], b["e"][:], didxt[:],
                        channels=128, num_elems=N * 128, num_idxs=N,
                    )
                    # one PSUM tile per bank so each drain half gates only
                    # on its own bank's matmuls
                    psh = [
                        ps.tile([128, 512], f32, tag=f"acc{h}",
                                name=f"ps{h}_{pp - 1}")
                        for h in range(2)
                    ]
                    # bank0 fully first so its drain can start while bank1
                    # still accumulates
                    tq = b["t"]
                    for h in range(2):
                        for n in range(N):
                            nc.tensor.matmul(
                                psh[h][:],
                                dgall[:, n * 128 : (n + 1) * 128],
                                tq[:, n * D + h * 512 : n * D + (h + 1) * 512],
                                start=(n == 0), stop=(n == N - 1),
                            )
                    b["ps"] = psh
                if pp < PB:
                    b = blk[pp]
                    # (GPSIMD tensor_mul here measured +42us: Q7 per-op
                    # latency on the critical softmax chain is brutal --
                    # keep GPSIMD strictly to the off-chain scatter)
                    nc.vector.tensor_mul(b["lg"][:], b["dot"][:], b["y0"][:])
                    nc.vector.tensor_reduce(
                        b["nm"][:], b["lg"][:], X, OP.max, negate=True
                    )
                if pp >= 2:
                    qp = pp - 2
                    b = blk.pop(qp)
                    acc = ac.tile([128, D], f16, tag="acc_sb")
                    psh = b["ps"]
                    # ACT-heavy drain split (measured balance); each half
                    # DMAs as soon as its drain lands
                    nc.scalar.activation(
                        acc[:, 0:512], psh[0][:], A.Copy, scale=b["rs"][:]
                    )
                    nc.sync.dma_start(
                        o[qp * 128 : (qp + 1) * 128, 0:512], acc[:, 0:512]
                    )
                    if qp % 2 == 1:
                        # odd blocks: second half on DVE (balances the
                        # engines; for the last block it also runs in
                        # parallel with ACT's first half, shortening the
                        # exposed tail)
                        nc.vector.tensor_scalar(
                            acc[:, 512:1024], psh[1][:],
                            b["rs"][:], None, OP.mult,
                        )
                    else:
                        nc.scalar.activation(
                            acc[:, 512:1024], psh[1][:],
                            A.Copy, scale=b["rs"][:],
                        )
                    nc.sync.dma_start(
                        o[qp * 128 : (qp + 1) * 128, 512:1024],
                        acc[:, 512:1024],
                    )

    nc.compile()
    return nc


def get_program():
    if "nc" not in _cache:
        _cache["nc"] = _build()
    return _cache["nc"]


def make_in_maps(V, proj, scale, block_idx):
    V = np.asarray(V, dtype=np.float32)
    proj = np.asarray(proj, dtype=np.float32)
    scale = np.asarray(scale, dtype=np.float32)
    idx = min(int(block_idx), proj.shape[0] - 1)
    ws = (proj[idx] * scale).astype(np.float16)
    wsb = np.ascontiguousarray(np.broadcast_to(ws, (128, D)))
    didx = (
        np.arange(N, dtype=np.int16)[None, :] * 128
        + np.arange(128, dtype=np.int16)[:, None]
    ).astype(np.int16)
    # [N, BS, D] -> [NCORES, PB, 128, N, D] fp16
    Vp = (
        V.reshape(N, NCORES, PB, 128, D)
        .transpose(1, 2, 3, 0, 4)
        .astype(np.float16)
    )
    return [
        {
            "v": np.ascontiguousarray(Vp[k]).reshape(PB, 128, ND),
            "wsb": wsb,
            "didx": didx,
        }
        for k in range(NCORES)
    ]


def kernel(V, proj, scale, block_idx):
    from concourse.bass_utils import run_bass_kernel_spmd

    nc = get_program()
    in_maps = make_in_maps(V, proj, scale, block_idx)
    res = run_bass_kernel_spmd(nc, in_maps, core_ids=list(range(NCORES)))
    _cache["last_exec_time_ns"] = res.exec_time_ns
    _cache["last_results"] = res
    out = np.concatenate(
        [res.results[k]["o"].astype(np.float32) for k in range(NCORES)], axis=0
    )
    return out.reshape(B, S, D)


# revision 44
# speedup vs baseline: 1.3910x; 1.0089x over previous
"""Trainium2 Bass kernel for nn_AttentionResidual (sparse_attention).

Computes, for V:(n=8,b=4,s=2048,d=1024), proj:(12,1024), scale:(1024,), block_idx:
    w       = proj[min(block_idx, 11)]
    rms     = sqrt(mean(V^2, axis=-1) + 1e-5)
    logits  = sum_d (w*scale)[d] * V[...,d] / rms
    weights = softmax(logits, axis=n)
    out     = sum_n weights[n] * V[n]                       # (b,s,d)

Sharding: data-parallel over the 8192 (b,s) positions across 8 NeuronCores
(1024 positions per core). proj/scale fold into one d-vector on the host.

Design (fp16 V in [block, pos, n, d] layout; one 2 MiB DMA per 128-position
block; ~112us measured vs 114.5us prior baseline). The kernel is bound by
the two free-axis reductions (ws-dot on DVE scalar_tensor_tensor+accum,
sum-of-squares mostly on ACT Square+accum), which no engine does faster
than ~1 elem/cycle/partition: DVE STT has no 2x modes; tensor_scalar
CACHE_REDUCE measures 1x on HW despite the cost model's 4x; TTR/bn_stats/
pool are all 1x; GPSIMD can neither reduce along the free axis (codegen
rejects TensorScalarPtr on Pool) nor touch PSUM, and putting even a tiny
[128,8] multiply on its Q7 cores costs +42us of chain latency. Two passes
over V are information-theoretically required (ss and dot are independent
functionals), so the ~86us/engine middle is the floor; measured balance:
58 ACT / 6 DVE sum-of-squares units + ACT-heavy PSUM drains lands ACT and
DVE both at ~86% busy. Fixed framework overheads bound the rest: ~6us
preamble (all-engine barriers + tpb_base loads) and ~6us exit (each
engine zeroes its ~51-semaphore pool one op at a time).
  - softmax stats on [128,8] tiles: ACT Ln/Exp (one table set with
    Square/Copy), DVE max/sum/recip; sume/rs for block q are interleaved
    behind the first DVE reduce unit of block q+1 so DVE never stalls
    waiting on ACT's Exp.
  - weighted sum on the TensorEngine: all 8 diag(e_n) built by a single
    GPSIMD local_scatter into a [128, 8*128] strip; 2x8 accumulating fp16
    matmuls per block. PSUM is TWO [128,512] tiles (one per bank) so each
    drain half waits only on its own bank's matmuls -- the tile framework
    gates readers on whole-tile writers, not overlapping subtiles.
  - PSUM drain (DMA cannot read PSUM): ACT Copy / DVE tensor_scalar with
    the 1/sum(e) softmax normalization folded into the per-partition
    scale; split ACT-heavy to balance the engines.
  - warmup: block 0's V arrives as 4 quarter DMAs. (An 8-separate-tile
    per-n split was tried to start compute earlier -- the ~6us startup
    is actually all-engine-barrier preamble, not DMA wait, and the split
    only added +/-1.5us of schedule variance. This 4-quarter form
    measures 113.3-113.5us with baseline-like ~0.1us repeatability.)
"""

import numpy as np

N, B, S, D = 8, 4, 2048, 1024
NCORES = 8
BS = B * S            # 8192 flattened (b,s) positions
PER = BS // NCORES    # 1024 positions per core
PB = PER // 128       # 8 position blocks per core
ND = N * D            # 8192 (n,d) elements per position
EPS = 1e-5

# Per-(block, n) engine for the sum-of-squares unit. A=ACT Square+accum,
# V=DVE STT+accum. (G=GPSIMD was tried: codegen rejects TensorScalarPtr
# on Pool -- GPSIMD cannot do free-axis reduces, period.) V units come
# FIRST in a block so the ACT Ln never waits on the tail of the DVE batch.
SOS_ENG = ["VAAAAAAA"] * 6 + ["AAAAAAAA"] * 2  # 58A/6V

_cache = {}


def _build():
    import concourse.tile as tile
    from concourse import bacc, mybir

    OP = mybir.AluOpType
    A = mybir.ActivationFunctionType
    X = mybir.AxisListType.X
    f32 = mybir.dt.float32
    f16 = mybir.dt.float16
    f8 = mybir.dt.float8e4

    from concourse.hw_specs import get_activation_tables

    nc = bacc.Bacc(
        "TRN2",
        target_bir_lowering=False,
        debug=False,
        enable_asserts=False,
        num_devices=NCORES,
        enable_partition_id=False,
    )
    v = nc.dram_tensor("v", [PB, 128, ND], f16, kind="ExternalInput").ap()
    wsb = nc.dram_tensor("wsb", [128, D], f16, kind="ExternalInput").ap()
    didx = nc.dram_tensor("didx", [128, N], mybir.dt.int16, kind="ExternalInput").ap()
    o = nc.dram_tensor("o", [PER, D], f16, kind="ExternalOutput").ap()

    # One ACT table set covers Square/Ln/Exp/Copy; pre-place its load so the
    # bacc pass doesn't ping-pong between smaller sets.
    act_set_id = list(get_activation_tables(nc.m.arch).keys()).index(
        "natural_log_exp_and_others"
    )

    with tile.TileContext(nc) as tc:
        with (
            tc.tile_pool(name="v0p", bufs=8) as v0p,
            tc.tile_pool(name="vp", bufs=4) as vp,
            tc.tile_pool(name="wp", bufs=1) as wp,
            tc.tile_pool(name="scrA", bufs=2) as scrA,
            tc.tile_pool(name="scrV", bufs=2) as scrV,
            tc.tile_pool(name="scrG", bufs=2) as scrG,
            tc.tile_pool(name="st", bufs=8) as st,
            tc.tile_pool(name="dg", bufs=3) as dgp,
            tc.tile_pool(name="ac", bufs=3) as ac,
            tc.tile_pool(name="ps", bufs=3, space="PSUM") as ps,
        ):
            nc.scalar.add_instruction(
                mybir.InstLoadActFuncSet(
                    name=nc.get_next_instruction_name(),
                    ins=[],
                    outs=[],
                    act_func_set_id=act_set_id,
                )
            )
            wt = wp.tile([128, D], f16, tag="w")
            didxt = wp.tile([128, N], mybir.dt.int16, tag="didx")
            epsb = wp.tile([128, 1], f32, tag="eps")
            nc.vector.memset(epsb[:], EPS)

            # Skewed software pipeline, one iteration per 128-position
            # block. In-order engine queues mean a dependency ping-pong
            # (ss -> Ln -> y0 -> lg -> nm -> e -> scatter -> matmul ->
            # drain) stalls every engine if issued densely per block;
            # instead each stage is issued one block behind the stage it
            # depends on, so every queued op's inputs are already complete
            # when reached:
            #   iter pp: ACT[e(pp-1)] DVE[sume,rs(pp-1) after 1st unit]
            #            reductions(pp) ACT[Ln,y0(pp)] DVE[lg,nm(pp)]
            #            Pool[scatter(pp-1)] PE[matmuls(pp-1)]
            #            ACT/DVE[drain(pp-2)] DMA[out(pp-2)]
            blk = {}

            def softmax_epilogue(qb):
                # sume/rs for block qb (DVE smalls feeding the drain scale)
                b = blk[qb]
                sume = st.tile([128, 1], f32, tag="sume", name=f"su_{qb}")
                nc.vector.tensor_reduce(sume[:], b["e"][:], X, OP.add)
                rs = st.tile([128, 1], f32, tag="rs", name=f"rs_{qb}")
                nc.vector.reciprocal(rs[:], sume[:])
                b["rs"] = rs

            for pp in range(PB + 2):
                if pp >= 1 and pp - 1 < PB:
                    b = blk[pp - 1]
                    e = st.tile([128, N], f16, tag="e", name=f"e_{pp - 1}")
                    nc.scalar.activation(
                        e[:], b["lg"][:], A.Exp, bias=b["nm"][:]
                    )
                    b["e"] = e
                    if pp >= PB:
                        # drain iteration: no reduction loop to interleave
                        # behind -- issue the epilogue directly
                        softmax_epilogue(pp - 1)
                if pp < PB:
                    sos_eng = SOS_ENG[pp]
                    t = vp.tile([128, ND], f16, tag="v", name=f"v_{pp}")
                    if pp == 0:
                        for q in range(4):
                            nc.sync.dma_start(
                                t[:, q * (ND // 4) : (q + 1) * (ND // 4)],
                                v[pp, :, q * (ND // 4) : (q + 1) * (ND // 4)],
                            )
                        nc.sync.dma_start(wt[:], wsb[:])
                        nc.sync.dma_start(didxt[:], didx[:])
                    else:
                        nc.sync.dma_start(t[:], v[pp, :, :])
                    ss = st.tile([128, N], f32, tag="ss", name=f"ss_{pp}")
                    dot = st.tile([128, N], f32, tag="dot", name=f"dot_{pp}")
                    for n in range(N):
                        vn = t[:, n * D : (n + 1) * D]
                        if sos_eng[n] == "A":
                            sq = scrA.tile([128, D], f8, tag="sqA")
                            nc.scalar.activation(
                                sq[:], vn, A.Square,
                                accum_out=ss[:, n : n + 1],
                            )
                        elif sos_eng[n] == "G":
                            sq = scrG.tile([128, D], f8, tag="sqG")
                            nc.gpsimd.scalar_tensor_tensor(
                                out=sq[:], in0=vn, scalar=1.0, in1=vn,
                                op0=OP.mult, op1=OP.mult,
                                accum_out=ss[:, n : n + 1],
                            )
                        else:
                            sq = scrV.tile([128, D], f8, tag="sqV")
                            nc.vector.scalar_tensor_tensor(
                                out=sq[:], in0=vn, scalar=1.0, in1=vn,
                                op0=OP.mult, op1=OP.mult,
                                accum_out=ss[:, n : n + 1],
                            )
                        td = scrV.tile([128, D], f8, tag="tdV")
                        nc.vector.scalar_tensor_tensor(
                            out=td[:], in0=vn, scalar=1.0, in1=wt[:],
                            op0=OP.mult, op1=OP.mult,
                            accum_out=dot[:, n : n + 1],
                        )
                        if n == 0 and pp >= 1:
                            # softmax epilogue of the previous block, issued
                            # behind the first DVE unit of this block so the
                            # ACT Exp above has landed by the time DVE gets
                            # here (no stall on the in-order queue)
                            softmax_epilogue(pp - 1)
                    lnt = st.tile([128, N], f32, tag="lnt", name=f"ln_{pp}")
                    nc.scalar.activation(
                        lnt[:], ss[:], A.Ln, bias=epsb[:], scale=1.0 / D
                    )
                    y0 = st.tile([128, N], f32, tag="y0", name=f"y0_{pp}")
                    nc.scalar.activation(y0[:], lnt[:], A.Exp, scale=-0.5)
                    blk[pp] = {
                        "t": t, "dot": dot, "y0": y0,
                        "lg": st.tile([128, N], f32, tag="lg", name=f"lg_{pp}"),
                        "nm": st.tile([128, 1], f32, tag="nm", name=f"nm_{pp}"),
                    }
                if pp >= 1 and pp - 1 < PB:
                    b = blk[pp - 1]
                    # diag strip built as TWO half-scatters into SEPARATE
                    # tiles (tile-granular gating): the n=0..3 matmuls can
                    # start after only half the scatter work. The second
                    # half reuses didxt[:,0:4] -- (n-4)*128+p == n'*128+p.
                    dga = dgp.tile([128, 512], f16, tag="dga",
                                   name=f"dga_{pp - 1}")
                    dgb = dgp.tile([128, 512], f16, tag="dgb",
                                   name=f"dgb_{pp - 1}")
                    nc.gpsimd.local_scatter(
                        dga[:], b["e"][:, 0:4], didxt[:, 0:4],
                        channels=128, num_elems=512, num_idxs=4,
                    )
                    nc.gpsimd.local_scatter(
                        dgb[:], b["e"][:, 4:8], didxt[:, 0:4],
                        channels=128, num_elems=512, num_idxs=4,
                    )
                    # one PSUM tile per bank so each drain half gates only
                    # on its own bank's matmuls
                    psh = [
                        ps.tile([128, 512], f32, tag=f"acc{h}",
                                name=f"ps{h}_{pp - 1}")
                        for h in range(2)
                    ]
                    # bank0 fully first so its drain can start while bank1
                    # still accumulates
                    tq = b["t"]
                    for h in range(2):
                        for n in range(N):
                            dgt = dga if n < 4 else dgb
                            nc.tensor.matmul(
                                psh[h][:],
                                dgt[:, (n % 4) * 128 : (n % 4 + 1) * 128],
                                tq[:, n * D + h * 512 : n * D + (h + 1) * 512],
                                start=(n == 0), stop=(n == N - 1),
                            )
                    b["ps"] = psh
                if pp < PB:
                    b = blk[pp]
                    # (GPSIMD tensor_mul here measured +42us: Q7 per-op
                    # latency on the critical softmax chain is brutal --
                    # keep GPSIMD strictly to the off-chain scatter)
                    nc.vector.tensor_mul(b["lg"][:], b["dot"][:], b["y0"][:])
                    nc.vector.tensor_reduce(
                        b["nm"][:], b["lg"][:], X, OP.max, negate=True
                    )
                if pp >= 2:
                    qp = pp - 2
                    b = blk.pop(qp)
                    acc = ac.tile([128, D], f16, tag="acc_sb")
                    psh = b["ps"]
                    # Drain split: DVE only takes halves for qp 1/3 (mid-
                    # stream balance); qp 5/6 go fully to ACT so the DVE
                    # queue runs straight into block 7's lg/nm instead of
                    # stalling the tail softmax behind a drain. Each piece
                    # DMAs as soon as its drain lands.
                    nc.scalar.activation(
                        acc[:, 0:512], psh[0][:], A.Copy, scale=b["rs"][:]
                    )
                    nc.sync.dma_start(
                        o[qp * 128 : (qp + 1) * 128, 0:512], acc[:, 0:512]
                    )
                    if qp == PB - 1:
                        # final block: bank1 as two quarter drains (ACT and
                        # DVE in parallel), each with its own DMA -- halves
                        # the exposed final-transfer latency (the last DMA
                        # is descriptor-latency-bound: 128 per-partition
                        # descriptors over 16 queues)
                        nc.scalar.activation(
                            acc[:, 512:768], psh[1][:, 0:256],
                            A.Copy, scale=b["rs"][:],
                        )
                        nc.sync.dma_start(
                            o[qp * 128 : (qp + 1) * 128, 512:768],
                            acc[:, 512:768],
                        )
                        nc.vector.tensor_scalar(
                            acc[:, 768:1024], psh[1][:, 256:512],
                            b["rs"][:], None, OP.mult,
                        )
                        nc.sync.dma_start(
                            o[qp * 128 : (qp + 1) * 128, 768:1024],
                            acc[:, 768:1024],
                        )
                    else:
                        if qp % 2 == 1 and qp < 5:
                            nc.vector.tensor_scalar(
                                acc[:, 512:1024], psh[1][:],
                                b["rs"][:], None, OP.mult,
                            )
                        else:
                            nc.scalar.activation(
                                acc[:, 512:1024], psh[1][:],
                                A.Copy, scale=b["rs"][:],
                            )
                        nc.sync.dma_start(
                            o[qp * 128 : (qp + 1) * 128, 512:1024],
                            acc[:, 512:1024],
                        )

    nc.compile()
    return nc


def get_program():
    if "nc" not in _cache:
        _cache["nc"] = _build()
    return _cache["nc"]


def make_in_maps(V, proj, scale, block_idx):
    V = np.asarray(V, dtype=np.float32)
    proj = np.asarray(proj, dtype=np.float32)
    scale = np.asarray(scale, dtype=np.float32)
    idx = min(int(block_idx), proj.shape[0] - 1)
    ws = (proj[idx] * scale).astype(np.float16)
    wsb = np.ascontiguousarray(np.broadcast_to(ws, (128, D)))
    didx = (
        np.arange(N, dtype=np.int16)[None, :] * 128
        + np.arange(128, dtype=np.int16)[:, None]
    ).astype(np.int16)
    # [N, BS, D] -> [NCORES, PB, 128, N, D] fp16
    Vp = (
        V.reshape(N, NCORES, PB, 128, D)
        .transpose(1, 2, 3, 0, 4)
        .astype(np.float16)
    )
    return [
        {
            "v": np.ascontiguousarray(Vp[k]).reshape(PB, 128, ND),
            "wsb": wsb,
            "didx": didx,
        }
        for k in range(NCORES)
    ]


def kernel(V, proj, scale, block_idx):
    from concourse.bass_utils import run_bass_kernel_spmd

    nc = get_program()
    in_maps = make_in_maps(V, proj, scale, block_idx)
    res = run_bass_kernel_spmd(nc, in_maps, core_ids=list(range(NCORES)))
    _cache["last_exec_time_ns"] = res.exec_time_ns
    _cache["last_results"] = res
    out = np.concatenate(
        [res.results[k]["o"].astype(np.float32) for k in range(NCORES)], axis=0
    )
    return out.reshape(B, S, D)
